# revision 3
# baseline (speedup 1.0000x reference)
"""Trainium2 Bass kernel for nn_NodeModel (GNN message passing).

Reference computation:
    agg = segment_sum(edge_attr, edge_index[1], num_segments=N)     # scatter-add
    h   = relu(concat([x, agg, u[batch]], 1) @ W1 + b1)
    out = h @ W2 + b2 + x

Strategy (8 NeuronCores, graph-parallel by destination node):
  - Nodes are padded to 100352 = 8 * 12544 and sharded contiguously across 8
    cores. Each core owns 12544 destination nodes = 49 ranges of 256 nodes.
  - Host groups edges by destination range (counting-sort), pads each range's
    edge list to a multiple of 128, and lays the per-core edge features out
    contiguously. Rebased destination columns (col % 256, pad = -1) ride along.
  - On device, the scatter-add is computed on the TensorEngine as a sequence of
    one-hot matmuls: for each 128-edge block, DVE builds a one-hot [128e, 256n]
    via tensor_scalar(is_equal) against an iota row; PE accumulates
    edge_blockT @ onehot into a PSUM bank per 256-node range, giving aggT
    [128 feat, 256 nodes] directly (no transposes needed downstream).
  - MLP runs per 512-node group: h1T[h,n] = W1x.T xT + W1a.T aggT + W1u.T ugT
    accumulated in PSUM, ReLU+bias on ScalarE during evacuation; layer 2
    produces natural-orientation out[n,d] with the residual (+x) folded in as
    an identity matmul from xT and the bias as a rank-1 matmul.
  - All matmul operands use float32r (TF32-like, 4-byte) — no casts needed.
"""

import os
from contextlib import ExitStack

import ml_dtypes
import numpy as np

N_NODES = 100000
N_EDGES = 1600000
D = 128          # node / edge feature dim
DG = 16          # global feature dim
H = 256          # hidden dim
G = 64           # graphs
NCORES = 8

NPC = 12544      # nodes per core (= 98 * 128 = 49 * 256)
N_PAD = NCORES * NPC
RW = 128         # scatter range width (nodes per PSUM accumulation group)
RPC = NPC // RW  # 98 ranges per core
EBLK = 128       # edges per matmul block
CHUNK_BLKS = 32  # edge blocks per DMA chunk (4096 edges = 2 MiB)

NB_MLP = 512     # nodes per MLP group

_PROFILE_RESULTS = [None]  # stash for test harness introspection


def _shard_inputs(x, edge_index, edge_attr, u, batch, W1, b1, W2, b2):
    x = np.ascontiguousarray(np.asarray(x, dtype=np.float32))
    edge_index = np.asarray(edge_index)
    edge_attr = np.ascontiguousarray(np.asarray(edge_attr, dtype=np.float32))
    u = np.asarray(u, dtype=np.float32)
    batch = np.asarray(batch)
    W1 = np.asarray(W1, dtype=np.float32)
    b1 = np.asarray(b1, dtype=np.float32)
    W2 = np.asarray(W2, dtype=np.float32)
    b2 = np.asarray(b2, dtype=np.float32)

    col = np.asarray(edge_index[1], dtype=np.int64)
    r_glob = (col // RW).astype(np.int64)           # global 256-node range id
    n_ranges = NCORES * RPC

    counts = np.bincount(r_glob, minlength=n_ranges)
    cnt_cl = counts.reshape(NCORES, RPC)
    # blocks per local range: shared across cores (same SPMD program)
    B = np.maximum(1, (cnt_cl.max(axis=0) + EBLK - 1) // EBLK).astype(np.int64)
    prefix = np.concatenate([[0], np.cumsum(B)])    # [RPC+1]
    nblk = int(prefix[-1])                          # blocks per core
    s_slots = nblk * EBLK
    nchunk = (s_slots + CHUNK_BLKS * EBLK - 1) // (CHUNK_BLKS * EBLK)
    s_alloc = nchunk * CHUNK_BLKS * EBLK
    nblk_alloc = s_alloc // EBLK

    order = np.argsort(r_glob, kind="stable")
    sorted_r = r_glob[order]
    starts = np.concatenate([[0], np.cumsum(counts)])[:-1]
    rank = np.arange(N_EDGES, dtype=np.int64) - starts[sorted_r]
    l_of = sorted_r % RPC
    core_of = sorted_r // RPC
    dst_slot = prefix[l_of] * EBLK + rank

    # swizzled edge layout: [core, chunk, p, blk_in_chunk, feat] so each chunk's
    # DMA is a fully contiguous [128, CHUNK_BLKS*128] 2D slice per partition
    blk_of = dst_slot // EBLK
    ea_all = np.zeros((NCORES, nchunk, EBLK, CHUNK_BLKS, D), dtype=np.float32)
    ea_all[core_of, blk_of // CHUNK_BLKS, dst_slot % EBLK, blk_of % CHUNK_BLKS] = (
        edge_attr[order]
    )
    ea_all = ea_all.reshape(NCORES, nchunk * EBLK, CHUNK_BLKS * D)
    colr_all = np.full((NCORES, s_alloc), -1.0, dtype=np.float32)
    colr_all[core_of, dst_slot] = (col[order] % RW).astype(np.float32)
    # [core, 128, nblk_alloc]: colrT[c, p, blk] = rebased col of edge slot blk*128+p
    colrT_all = np.ascontiguousarray(
        colr_all.reshape(NCORES, nblk_alloc, EBLK).transpose(0, 2, 1)
    )

    x_pad = np.zeros((N_PAD, D), dtype=np.float32)
    x_pad[:N_NODES] = x
    xT_all = np.ascontiguousarray(x_pad.reshape(NCORES, NPC, D).transpose(0, 2, 1))

    batch_pad = np.concatenate(
        [batch, np.full(N_PAD - N_NODES, batch[-1], dtype=batch.dtype)]
    ).astype(np.int64)
    ug = u[batch_pad]                                # [N_PAD, DG]
    ugT_all = np.ascontiguousarray(ug.reshape(NCORES, NPC, DG).transpose(0, 2, 1))

    consts = {
        "w1x": np.ascontiguousarray(W1[:D]),                  # [128, 256]
        "w1a": np.ascontiguousarray(W1[D : 2 * D]),           # [128, 256]
        "w1u": np.ascontiguousarray(W1[2 * D :]),             # [16, 256]
        "b1t": np.ascontiguousarray(b1.reshape(2, D).T),      # [128, 2]
        "w2a": np.ascontiguousarray(W2[:D]),                  # [128, 128]
        "w2b": np.ascontiguousarray(W2[D:]),                  # [128, 128]
        "b2r": np.ascontiguousarray(b2[None, :]),             # [1, 128]
        "ones": np.ones((1, NB_MLP), dtype=np.float32),
        "ident": np.eye(D, dtype=np.float32),
        "iota": np.tile(np.arange(RW, dtype=np.float32), (D, 1)).astype(ml_dtypes.bfloat16),
    }

    in_maps = []
    for c in range(NCORES):
        m = {
            "ea": ea_all[c],
            "colrt": colrT_all[c],
            "xt": xT_all[c],
            "ugt": ugT_all[c],
        }
        m.update(consts)
        in_maps.append(m)
    return in_maps, B, nchunk, nblk_alloc


def _build_program(B, nchunk, nblk_alloc, loop_n=1, variant="full"):
    import concourse.bacc as bacc
    import concourse.mybir as mybir
    import concourse.tile as tile

    F32 = mybir.dt.float32
    F32R = mybir.dt.float32r
    BF16 = mybir.dt.bfloat16
    s_alloc = nchunk * CHUNK_BLKS * EBLK
    prefix = np.concatenate([[0], np.cumsum(B)])

    nc = bacc.Bacc("TRN2", target_bir_lowering=False, debug=False)

    ea_d = nc.dram_tensor("ea", [nchunk * EBLK, CHUNK_BLKS * D], F32,
                          kind="ExternalInput")
    colrt_d = nc.dram_tensor("colrt", [EBLK, nblk_alloc], F32, kind="ExternalInput")
    xt_d = nc.dram_tensor("xt", [D, NPC], F32R, kind="ExternalInput")
    ugt_d = nc.dram_tensor("ugt", [DG, NPC], F32R, kind="ExternalInput")
    w1x_d = nc.dram_tensor("w1x", [D, H], F32R, kind="ExternalInput")
    w1a_d = nc.dram_tensor("w1a", [D, H], F32R, kind="ExternalInput")
    w1u_d = nc.dram_tensor("w1u", [DG, H], F32R, kind="ExternalInput")
    b1t_d = nc.dram_tensor("b1t", [D, 2], F32, kind="ExternalInput")
    w2a_d = nc.dram_tensor("w2a", [D, D], F32R, kind="ExternalInput")
    w2b_d = nc.dram_tensor("w2b", [D, D], F32R, kind="ExternalInput")
    b2r_d = nc.dram_tensor("b2r", [1, D], F32R, kind="ExternalInput")
    ones_d = nc.dram_tensor("ones", [1, NB_MLP], F32R, kind="ExternalInput")
    ident_d = nc.dram_tensor("ident", [D, D], F32R, kind="ExternalInput")
    iota_d = nc.dram_tensor("iota", [D, RW], BF16, kind="ExternalInput")
    out_d = nc.dram_tensor("out", [NPC, D], F32, kind="ExternalOutput")

    import contextlib

    with tile.TileContext(nc) as tc, ExitStack() as ctx:
        persist = ctx.enter_context(tc.tile_pool(name="persist", bufs=1))
        ea_pool = ctx.enter_context(tc.tile_pool(name="ea", bufs=3))
        eabf_pool = ctx.enter_context(tc.tile_pool(name="eabf", bufs=3))
        oh_pool = ctx.enter_context(tc.tile_pool(name="oh", bufs=24))
        agg_pool = ctx.enter_context(tc.tile_pool(name="agg", bufs=8))
        ug_pool = ctx.enter_context(tc.tile_pool(name="ug", bufs=2))
        hs_pool = ctx.enter_context(tc.tile_pool(name="hs", bufs=4))
        os_pool = ctx.enter_context(tc.tile_pool(name="os", bufs=2))
        o2sb_pool = ctx.enter_context(tc.tile_pool(name="o2sb", bufs=2))
        sc_psum = ctx.enter_context(tc.tile_pool(name="scps", bufs=2, space="PSUM"))
        h_psum = ctx.enter_context(tc.tile_pool(name="hps", bufs=2, space="PSUM"))
        o2_psum = ctx.enter_context(tc.tile_pool(name="o2ps", bufs=2, space="PSUM"))
        t_psum = ctx.enter_context(tc.tile_pool(name="tps", bufs=2, space="PSUM"))

        # --- persistent loads -------------------------------------------------
        def pload(dram, shape, dtype, engine):
            t = persist.tile(shape, dtype, tag=dram.name)
            engine.dma_start(t[:], dram.ap())
            return t

        w1x_t = pload(w1x_d, [D, H], F32R, nc.scalar)
        w1a_t = pload(w1a_d, [D, H], F32R, nc.scalar)
        w1u_t = pload(w1u_d, [DG, H], F32R, nc.scalar)
        b1t_t = pload(b1t_d, [D, 2], F32, nc.scalar)
        w2a_t = pload(w2a_d, [D, D], F32R, nc.scalar)
        w2b_t = pload(w2b_d, [D, D], F32R, nc.scalar)
        b2r_t = pload(b2r_d, [1, D], F32R, nc.scalar)
        ones_t = pload(ones_d, [1, NB_MLP], F32R, nc.scalar)
        ident_t = pload(ident_d, [D, D], F32R, nc.scalar)
        iota_t = pload(iota_d, [D, RW], BF16, nc.scalar)
        colrt_t = pload(colrt_d, [EBLK, nblk_alloc], F32, nc.scalar)
        xt_t = pload(xt_d, [D, NPC], F32R, nc.scalar)

        chunk_tiles = {}

        def get_chunk(ci):
            if ci not in chunk_tiles:
                if variant == "dmacast":
                    t = eabf_pool.tile([EBLK, CHUNK_BLKS * D], BF16, tag="eabf",
                                       name="eabf", bufs=6)
                    nc.gpsimd.dma_start(
                        t[:], ea_d.ap()[ci * EBLK : (ci + 1) * EBLK, :]
                    )
                else:
                    t32 = ea_pool.tile([EBLK, CHUNK_BLKS * D], F32, tag="eachunk")
                    nc.sync.dma_start(
                        t32[:], ea_d.ap()[ci * EBLK : (ci + 1) * EBLK, :]
                    )
                    t = eabf_pool.tile([EBLK, CHUNK_BLKS * D], BF16, tag="eabf")
                    nc.scalar.copy(t[:], t32[:])
                chunk_tiles[ci] = t
            return chunk_tiles[ci]

        agg_tiles = [None] * (RPC // 2 + 1)

        oh_shared = [None]

        def scatter_range(l):
            ps = sc_psum.tile([D, RW], F32, tag="scps")
            nb = int(B[l])
            for b in range(nb):
                blk = int(prefix[l]) + b
                ea_t = get_chunk(blk // CHUNK_BLKS)
                co = blk % CHUNK_BLKS
                if variant == "noheq":
                    if oh_shared[0] is None:
                        oh = oh_pool.tile([EBLK, RW], BF16, tag="oh")
                        nc.vector.tensor_scalar(
                            oh[:], iota_t[:], colrt_t[:, 0:1], None,
                            mybir.AluOpType.is_equal,
                        )
                        oh_shared[0] = oh
                    oh = oh_shared[0]
                else:
                    oh = oh_pool.tile([EBLK, RW], BF16, tag="oh")
                    nc.vector.tensor_scalar(
                        oh[:],
                        iota_t[:],
                        colrt_t[:, blk : blk + 1],
                        None,
                        mybir.AluOpType.is_equal,
                    )
                if variant == "nomm":
                    if b == 0:
                        nc.tensor.matmul(ps[:], ea_t[:, co * D : (co + 1) * D],
                                         oh[:], start=True, stop=True)
                else:
                    nc.tensor.matmul(
                        ps[:],
                        ea_t[:, co * D : (co + 1) * D],
                        oh[:],
                        start=(b == 0),
                        stop=(b == nb - 1),
                    )
            # pack two 128-node ranges into one [128, 256] agg tile so the
            # MLP agg-term matmul keeps N=256 (f32r 1 cyc/row)
            if l % 2 == 0:
                agg_tiles[l // 2] = agg_pool.tile([D, 2 * RW], F32R, tag="agg", name="aggp")
            at = agg_tiles[l // 2]
            nc.scalar.copy(at[:, (l % 2) * RW : (l % 2 + 1) * RW], ps[:])

        Relu = mybir.ActivationFunctionType.Relu

        def mlp_group(g):
            gs = g * NB_MLP
            nb = min(NB_MLP, NPC - gs)
            pairs = [j for j in (2 * g, 2 * g + 1) if j * 2 * RW < gs + nb]
            ug_t = ug_pool.tile([DG, nb], F32R, tag="ug")
            nc.scalar.dma_start(ug_t[:], ugt_d.ap()[:, gs : gs + nb])
            hs = []
            for ht in range(2):
                hp = h_psum.tile([D, nb], F32, tag="hps")
                hsl = slice(ht * D, (ht + 1) * D)
                nc.tensor.matmul(
                    hp[:], w1x_t[:, hsl], xt_t[:, gs : gs + nb], start=True, stop=False
                )
                for j in pairs:
                    o0 = j * 2 * RW - gs
                    nc.tensor.matmul(
                        hp[:, o0 : o0 + 2 * RW],
                        w1a_t[:, hsl],
                        agg_tiles[j][:],
                        start=False,
                        stop=False,
                    )
                nc.tensor.matmul(
                    hp[:], w1u_t[:, hsl], ug_t[:], start=False, stop=True
                )
                ht_sb = hs_pool.tile([D, nb], F32R, tag="hs")
                nc.scalar.activation(
                    ht_sb[:], hp[:], Relu, bias=b1t_t[:, ht : ht + 1]
                )
                hs.append(ht_sb)
            # layer 2 in transposed orientation: o2T[d, n], N=nb (f32r 1 cyc/row)
            o2 = o2_psum.tile([D, nb], F32, tag="o2ps")
            nc.tensor.matmul(o2[:], w2a_t[:], hs[0][:], start=True, stop=False)
            nc.tensor.matmul(o2[:], w2b_t[:], hs[1][:], start=False, stop=False)
            # residual: += I.T @ xT = xT
            nc.tensor.matmul(o2[:], ident_t[:], xt_t[:, gs : gs + nb],
                             start=False, stop=False)
            # bias: += b2[d] * ones[n]  (rank-1)
            nc.tensor.matmul(o2[:], b2r_t[:], ones_t[:, :nb], start=False, stop=True)
            o2_sb = o2sb_pool.tile([D, nb], F32R, tag="o2sb")
            nc.scalar.copy(o2_sb[:], o2[:])
            # transpose back to natural [n, d] via PE, 128 nodes at a time
            o_sb = os_pool.tile([D, nb], F32, tag="os")
            for nt in range(nb // D):
                nsl = slice(nt * D, (nt + 1) * D)
                tp = t_psum.tile([D, D], F32R, tag="tps")
                nc.tensor.transpose(tp[:], o2_sb[:, nsl], ident_t[:])
                nc.scalar.copy(o_sb[:, nsl], tp[:])
            dst = out_d.ap()[gs : gs + nb, :].rearrange("(b p) f -> p b f", p=EBLK)
            nc.scalar.dma_start(dst, o_sb[:].rearrange("p (b f) -> p b f", f=D))

        ngrp = (NPC + NB_MLP - 1) // NB_MLP
        loop_cm = tc.For_i(0, loop_n, 1) if loop_n > 1 else contextlib.nullcontext()
        with loop_cm:
            if variant == "dmaonly":
                dummy = persist.tile([EBLK, D], F32, tag="dummy")
                for ci in range(nchunk):
                    t = get_chunk(ci)
                    nc.vector.tensor_copy(dummy[:], t[:, 0:D].bitcast(F32))
                chunk_tiles.clear()
            else:
                for g in range(ngrp):
                    for l in (4 * g, 4 * g + 1, 4 * g + 2, 4 * g + 3):
                        if l < RPC:
                            scatter_range(l)
                    mlp_group(g)

    nc.compile()
    return nc


def kernel(**inputs) -> np.ndarray:
    in_maps, B, nchunk, nblk_alloc = _shard_inputs(
        inputs["x"], inputs["edge_index"], inputs["edge_attr"], inputs["u"],
        inputs["batch"], inputs["W1"], inputs["b1"], inputs["W2"], inputs["b2"],
    )
    nc = _build_program(B, nchunk, nblk_alloc)

    from concourse.bass_utils import run_bass_kernel_spmd

    want_trace = bool(os.environ.get("KPROF"))
    if want_trace:
        try:
            from antenv.axon_hooks import get_axon_ntff_profile_hook  # noqa: F401
        except ImportError:
            want_trace = False
    res = run_bass_kernel_spmd(nc, in_maps, list(range(NCORES)), trace=want_trace)
    _PROFILE_RESULTS[0] = res
    out = np.concatenate([res.results[c]["out"] for c in range(NCORES)], axis=0)
    return np.ascontiguousarray(out[:N_NODES])



# revision 10
# speedup vs baseline: 1.4699x; 1.4699x over previous
"""Trainium2 Bass kernel for nn_NodeModel (GNN message passing).

Reference computation:
    agg = segment_sum(edge_attr, edge_index[1], num_segments=N)     # scatter-add
    h   = relu(concat([x, agg, u[batch]], 1) @ W1 + b1)
    out = h @ W2 + b2 + x

Strategy (8 NeuronCores, graph-parallel by destination node):
  - Nodes are padded to 100352 = 8 * 12544 and sharded contiguously across 8
    cores. Each core owns 12544 destination nodes = 49 ranges of 256 nodes.
  - Host groups edges by destination range (counting-sort), pads each range's
    edge list to a multiple of 128, and lays the per-core edge features out
    contiguously. Rebased destination columns (col % 256, pad = -1) ride along.
  - On device, the scatter-add is computed on the TensorEngine as a sequence of
    one-hot matmuls: for each 128-edge block, DVE builds a one-hot [128e, 256n]
    via tensor_scalar(is_equal) against an iota row; PE accumulates
    edge_blockT @ onehot into a PSUM bank per 256-node range, giving aggT
    [128 feat, 256 nodes] directly (no transposes needed downstream).
  - MLP runs per 512-node group: h1T[h,n] = W1x.T xT + W1a.T aggT + W1u.T ugT
    accumulated in PSUM, ReLU+bias on ScalarE during evacuation; layer 2
    produces natural-orientation out[n,d] with the residual (+x) folded in as
    an identity matmul from xT and the bias as a rank-1 matmul.
  - All matmul operands use float32r (TF32-like, 4-byte) — no casts needed.
"""

import os
from contextlib import ExitStack

import ml_dtypes
import numpy as np

N_NODES = 100000
N_EDGES = 1600000
D = 128          # node / edge feature dim
DG = 16          # global feature dim
H = 256          # hidden dim
G = 64           # graphs
NCORES = 8

NPC = 12544      # nodes per core (= 98 * 128 = 49 * 256)
N_PAD = NCORES * NPC
RW = 128         # scatter range width (nodes per PSUM accumulation group)
RPC = NPC // RW  # 98 ranges per core
EBLK = 128       # edges per matmul block
CHUNK_BLKS = 32  # edge blocks per DMA chunk (4096 edges = 2 MiB)

NB_MLP = 512     # nodes per MLP group

_PROFILE_RESULTS = [None]  # stash for test harness introspection


def _shard_inputs(x, edge_index, edge_attr, u, batch, W1, b1, W2, b2):
    x = np.ascontiguousarray(np.asarray(x, dtype=np.float32))
    edge_index = np.asarray(edge_index)
    edge_attr = np.ascontiguousarray(np.asarray(edge_attr, dtype=np.float32))
    u = np.asarray(u, dtype=np.float32)
    batch = np.asarray(batch)
    W1 = np.asarray(W1, dtype=np.float32)
    b1 = np.asarray(b1, dtype=np.float32)
    W2 = np.asarray(W2, dtype=np.float32)
    b2 = np.asarray(b2, dtype=np.float32)

    col = np.asarray(edge_index[1], dtype=np.int64)
    r_glob = (col // RW).astype(np.int64)           # global range id
    n_ranges = NCORES * RPC

    counts = np.bincount(r_glob, minlength=n_ranges)
    cnt_cl = counts.reshape(NCORES, RPC)
    # blocks per local range: shared across cores (same SPMD program)
    B = np.maximum(1, (cnt_cl.max(axis=0) + EBLK - 1) // EBLK).astype(np.int64)
    prefix = np.concatenate([[0], np.cumsum(B)])    # [RPC+1]
    nblk = int(prefix[-1])                          # blocks per core
    s_slots = nblk * EBLK
    nchunk = (s_slots + CHUNK_BLKS * EBLK - 1) // (CHUNK_BLKS * EBLK)
    s_alloc = nchunk * CHUNK_BLKS * EBLK
    nblk_alloc = s_alloc // EBLK

    order = np.argsort(r_glob, kind="stable")
    sorted_r = r_glob[order]
    starts = np.concatenate([[0], np.cumsum(counts)])[:-1]
    rank = np.arange(N_EDGES, dtype=np.int64) - starts[sorted_r]
    l_of = sorted_r % RPC
    core_of = sorted_r // RPC
    dst_slot = prefix[l_of] * EBLK + rank

    # swizzled edge layout: [core, chunk, p, blk_in_chunk, feat] so each chunk's
    # DMA is a fully contiguous [128, CHUNK_BLKS*128] 2D slice per partition.
    # Stored bf16: the scatter matmul consumes bf16, so cast host-side instead
    # of burning ScalarE on-device (and halve the dominant HBM stream).
    blk_of = dst_slot // EBLK
    ea_all = np.zeros((NCORES, nchunk, EBLK, CHUNK_BLKS, D), dtype=ml_dtypes.bfloat16)
    ea_all[core_of, blk_of // CHUNK_BLKS, dst_slot % EBLK, blk_of % CHUNK_BLKS] = (
        edge_attr[order].astype(ml_dtypes.bfloat16)
    )
    ea_all = ea_all.reshape(NCORES, nchunk * EBLK, CHUNK_BLKS * D)
    colr_all = np.full((NCORES, s_alloc), -1.0, dtype=np.float32)
    colr_all[core_of, dst_slot] = (col[order] % RW).astype(np.float32)
    # [core, 128, nblk_alloc]: colrT[c, p, blk] = rebased col of edge slot blk*128+p
    colrT_all = np.ascontiguousarray(
        colr_all.reshape(NCORES, nblk_alloc, EBLK).transpose(0, 2, 1)
    )

    x_pad = np.zeros((N_PAD, D), dtype=np.float32)
    x_pad[:N_NODES] = x
    xT_all = np.ascontiguousarray(x_pad.reshape(NCORES, NPC, D).transpose(0, 2, 1))

    batch_pad = np.concatenate(
        [batch, np.full(N_PAD - N_NODES, batch[-1], dtype=batch.dtype)]
    ).astype(np.int64)
    ug = u[batch_pad]                                # [N_PAD, DG]
    ugT_all = np.ascontiguousarray(ug.reshape(NCORES, NPC, DG).transpose(0, 2, 1))

    consts = {
        "w1x": np.ascontiguousarray(W1[:D]),                  # [128, 256]
        "w1a": np.ascontiguousarray(W1[D : 2 * D]),           # [128, 256]
        "w1u": np.ascontiguousarray(W1[2 * D :]),             # [16, 256]
        "b1t": np.ascontiguousarray(b1.reshape(2, D).T),      # [128, 2]
        "w2a": np.ascontiguousarray(W2[:D]),                  # [128, 128]
        "w2b": np.ascontiguousarray(W2[D:]),                  # [128, 128]
        "b2r": np.ascontiguousarray(b2[None, :]),             # [1, 128]
        "ones": np.ones((1, NB_MLP), dtype=np.float32),
        "ident": np.eye(D, dtype=np.float32),
        "iota": np.tile(np.arange(RW, dtype=np.float32), (D, 1)).astype(ml_dtypes.bfloat16),
    }

    in_maps = []
    for c in range(NCORES):
        m = {
            "ea": ea_all[c],
            "colrt": colrT_all[c],
            "xt": xT_all[c],
            "ugt": ugT_all[c],
        }
        m.update(consts)
        in_maps.append(m)
    return in_maps, B, nchunk, nblk_alloc


def _build_program(B, nchunk, nblk_alloc, loop_n=1, variant="full"):
    import concourse.bacc as bacc
    import concourse.mybir as mybir
    import concourse.tile as tile

    F32 = mybir.dt.float32
    F32R = mybir.dt.float32r
    BF16 = mybir.dt.bfloat16
    s_alloc = nchunk * CHUNK_BLKS * EBLK
    prefix = np.concatenate([[0], np.cumsum(B)])

    nc = bacc.Bacc("TRN2", target_bir_lowering=False, debug=False)

    ea_d = nc.dram_tensor("ea", [nchunk * EBLK, CHUNK_BLKS * D], BF16,
                          kind="ExternalInput")
    colrt_d = nc.dram_tensor("colrt", [EBLK, nblk_alloc], F32, kind="ExternalInput")
    xt_d = nc.dram_tensor("xt", [D, NPC], F32R, kind="ExternalInput")
    ugt_d = nc.dram_tensor("ugt", [DG, NPC], F32R, kind="ExternalInput")
    w1x_d = nc.dram_tensor("w1x", [D, H], F32R, kind="ExternalInput")
    w1a_d = nc.dram_tensor("w1a", [D, H], F32R, kind="ExternalInput")
    w1u_d = nc.dram_tensor("w1u", [DG, H], F32R, kind="ExternalInput")
    b1t_d = nc.dram_tensor("b1t", [D, 2], F32, kind="ExternalInput")
    w2a_d = nc.dram_tensor("w2a", [D, D], F32R, kind="ExternalInput")
    w2b_d = nc.dram_tensor("w2b", [D, D], F32R, kind="ExternalInput")
    b2r_d = nc.dram_tensor("b2r", [1, D], F32R, kind="ExternalInput")
    ones_d = nc.dram_tensor("ones", [1, NB_MLP], F32R, kind="ExternalInput")
    ident_d = nc.dram_tensor("ident", [D, D], F32R, kind="ExternalInput")
    iota_d = nc.dram_tensor("iota", [D, RW], BF16, kind="ExternalInput")
    out_d = nc.dram_tensor("out", [NPC, D], F32, kind="ExternalOutput")

    import contextlib

    with tile.TileContext(nc) as tc, ExitStack() as ctx:
        persist = ctx.enter_context(tc.tile_pool(name="persist", bufs=1))
        eabf_pool = ctx.enter_context(tc.tile_pool(name="eabf", bufs=4))
        oh_pool = ctx.enter_context(tc.tile_pool(name="oh", bufs=24))
        agg_pool = ctx.enter_context(tc.tile_pool(name="agg", bufs=8))
        ug_pool = ctx.enter_context(tc.tile_pool(name="ug", bufs=2))
        hs_pool = ctx.enter_context(tc.tile_pool(name="hs", bufs=4))
        os_pool = ctx.enter_context(tc.tile_pool(name="os", bufs=2))
        o2sb_pool = ctx.enter_context(tc.tile_pool(name="o2sb", bufs=2))
        sc_psum = ctx.enter_context(tc.tile_pool(name="scps", bufs=2, space="PSUM"))
        h_psum = ctx.enter_context(tc.tile_pool(name="hps", bufs=2, space="PSUM"))
        o2_psum = ctx.enter_context(tc.tile_pool(name="o2ps", bufs=2, space="PSUM"))
        t_psum = ctx.enter_context(tc.tile_pool(name="tps", bufs=2, space="PSUM"))

        # --- persistent loads -------------------------------------------------
        def pload(dram, shape, dtype, engine):
            t = persist.tile(shape, dtype, tag=dram.name)
            engine.dma_start(t[:], dram.ap())
            return t

        w1x_t = pload(w1x_d, [D, H], F32R, nc.scalar)
        w1a_t = pload(w1a_d, [D, H], F32R, nc.scalar)
        w1u_t = pload(w1u_d, [DG, H], F32R, nc.scalar)
        b1t_t = pload(b1t_d, [D, 2], F32, nc.scalar)
        w2a_t = pload(w2a_d, [D, D], F32R, nc.scalar)
        w2b_t = pload(w2b_d, [D, D], F32R, nc.scalar)
        b2r_t = pload(b2r_d, [1, D], F32R, nc.scalar)
        ones_t = pload(ones_d, [1, NB_MLP], F32R, nc.scalar)
        ident_t = pload(ident_d, [D, D], F32R, nc.scalar)
        iota_t = pload(iota_d, [D, RW], BF16, nc.scalar)
        colrt_t = pload(colrt_d, [EBLK, nblk_alloc], F32, nc.scalar)
        xt_t = pload(xt_d, [D, NPC], F32R, nc.scalar)

        chunk_tiles = {}

        def get_chunk(ci):
            if ci not in chunk_tiles:
                t = eabf_pool.tile([EBLK, CHUNK_BLKS * D], BF16, tag="eabf")
                nc.sync.dma_start(
                    t[:], ea_d.ap()[ci * EBLK : (ci + 1) * EBLK, :]
                )
                chunk_tiles[ci] = t
            return chunk_tiles[ci]

        agg_tiles = [None] * (RPC // 2 + 1)

        oh_shared = [None]

        def scatter_range(l):
            ps = sc_psum.tile([D, RW], F32, tag="scps")
            nb = int(B[l])
            for b in range(nb):
                blk = int(prefix[l]) + b
                ea_t = get_chunk(blk // CHUNK_BLKS)
                co = blk % CHUNK_BLKS
                if variant == "noheq":
                    if oh_shared[0] is None:
                        oh = oh_pool.tile([EBLK, RW], BF16, tag="oh")
                        nc.vector.tensor_scalar(
                            oh[:], iota_t[:], colrt_t[:, 0:1], None,
                            mybir.AluOpType.is_equal,
                        )
                        oh_shared[0] = oh
                    oh = oh_shared[0]
                else:
                    oh = oh_pool.tile([EBLK, RW], BF16, tag="oh")
                    nc.vector.tensor_scalar(
                        oh[:],
                        iota_t[:],
                        colrt_t[:, blk : blk + 1],
                        None,
                        mybir.AluOpType.is_equal,
                    )
                if variant == "nomm":
                    if b == 0:
                        nc.tensor.matmul(ps[:], ea_t[:, co * D : (co + 1) * D],
                                         oh[:], start=True, stop=True)
                else:
                    nc.tensor.matmul(
                        ps[:],
                        ea_t[:, co * D : (co + 1) * D],
                        oh[:],
                        start=(b == 0),
                        stop=(b == nb - 1),
                    )
            # pack two 128-node ranges into one [128, 256] agg tile so the
            # MLP agg-term matmul keeps N=256 (f32r 1 cyc/row)
            if l % 2 == 0:
                agg_tiles[l // 2] = agg_pool.tile([D, 2 * RW], F32R, tag="agg", name="aggp")
            at = agg_tiles[l // 2]
            nc.scalar.copy(at[:, (l % 2) * RW : (l % 2 + 1) * RW], ps[:])

        Relu = mybir.ActivationFunctionType.Relu

        def mlp_group(g):
            gs = g * NB_MLP
            nb = min(NB_MLP, NPC - gs)
            pairs = [j for j in (2 * g, 2 * g + 1) if j * 2 * RW < gs + nb]
            ug_t = ug_pool.tile([DG, nb], F32R, tag="ug")
            nc.scalar.dma_start(ug_t[:], ugt_d.ap()[:, gs : gs + nb])
            hs = []
            for ht in range(2):
                hp = h_psum.tile([D, nb], F32, tag="hps")
                hsl = slice(ht * D, (ht + 1) * D)
                nc.tensor.matmul(
                    hp[:], w1x_t[:, hsl], xt_t[:, gs : gs + nb], start=True, stop=False
                )
                for j in pairs:
                    o0 = j * 2 * RW - gs
                    nc.tensor.matmul(
                        hp[:, o0 : o0 + 2 * RW],
                        w1a_t[:, hsl],
                        agg_tiles[j][:],
                        start=False,
                        stop=False,
                    )
                nc.tensor.matmul(
                    hp[:], w1u_t[:, hsl], ug_t[:], start=False, stop=True
                )
                ht_sb = hs_pool.tile([D, nb], F32R, tag="hs")
                nc.scalar.activation(
                    ht_sb[:], hp[:], Relu, bias=b1t_t[:, ht : ht + 1]
                )
                hs.append(ht_sb)
            # layer 2 in transposed orientation: o2T[d, n], N=nb (f32r 1 cyc/row)
            o2 = o2_psum.tile([D, nb], F32, tag="o2ps")
            nc.tensor.matmul(o2[:], w2a_t[:], hs[0][:], start=True, stop=False)
            nc.tensor.matmul(o2[:], w2b_t[:], hs[1][:], start=False, stop=False)
            # residual: += I.T @ xT = xT
            nc.tensor.matmul(o2[:], ident_t[:], xt_t[:, gs : gs + nb],
                             start=False, stop=False)
            # bias: += b2[d] * ones[n]  (rank-1)
            nc.tensor.matmul(o2[:], b2r_t[:], ones_t[:, :nb], start=False, stop=True)
            o2_sb = o2sb_pool.tile([D, nb], F32R, tag="o2sb")
            nc.scalar.copy(o2_sb[:], o2[:])
            # transpose back to natural [n, d] via PE, 128 nodes at a time
            o_sb = os_pool.tile([D, nb], F32, tag="os")
            for nt in range(nb // D):
                nsl = slice(nt * D, (nt + 1) * D)
                tp = t_psum.tile([D, D], F32R, tag="tps")
                nc.tensor.transpose(tp[:], o2_sb[:, nsl], ident_t[:])
                nc.scalar.copy(o_sb[:, nsl], tp[:])
            dst = out_d.ap()[gs : gs + nb, :].rearrange("(b p) f -> p b f", p=EBLK)
            nc.scalar.dma_start(dst, o_sb[:].rearrange("p (b f) -> p b f", f=D))

        ngrp = (NPC + NB_MLP - 1) // NB_MLP
        loop_cm = tc.For_i(0, loop_n, 1) if loop_n > 1 else contextlib.nullcontext()
        with loop_cm:
            if variant == "dmaonly":
                dummy = persist.tile([EBLK, D], F32, tag="dummy")
                for ci in range(nchunk):
                    t = get_chunk(ci)
                    nc.vector.tensor_copy(dummy[:], t[:, 0:D].bitcast(F32))
                chunk_tiles.clear()
            else:
                for g in range(ngrp):
                    for l in (4 * g, 4 * g + 1, 4 * g + 2, 4 * g + 3):
                        if l < RPC:
                            scatter_range(l)
                    mlp_group(g)

    nc.compile()
    return nc


def kernel(**inputs) -> np.ndarray:
    in_maps, B, nchunk, nblk_alloc = _shard_inputs(
        inputs["x"], inputs["edge_index"], inputs["edge_attr"], inputs["u"],
        inputs["batch"], inputs["W1"], inputs["b1"], inputs["W2"], inputs["b2"],
    )
    nc = _build_program(B, nchunk, nblk_alloc)

    from concourse.bass_utils import run_bass_kernel_spmd

    want_trace = bool(os.environ.get("KPROF"))
    if want_trace:
        try:
            from antenv.axon_hooks import get_axon_ntff_profile_hook  # noqa: F401
        except ImportError:
            want_trace = False
    res = run_bass_kernel_spmd(nc, in_maps, list(range(NCORES)), trace=want_trace)
    _PROFILE_RESULTS[0] = res
    out = np.concatenate([res.results[c]["out"] for c in range(NCORES)], axis=0)
    return np.ascontiguousarray(out[:N_NODES])

